# revision 12
# baseline (speedup 1.0000x reference)
"""Trainium2 Bass kernel for the Capsule routing module (nn_Capsule_2224793059594).

Full inputs in, full output out. Data-parallel over batch: 32 batches -> 8
cores x 4 batches. Per core:

  Projection (PE): u_hat[b, i, (n,d)] is produced directly in the
  "T layout" [n(part), (b, i, d)] via 64 per-d matmuls:
     psum_d[n, (b,i)] = sum_k Kmat[k, (n,d)]^T . uT[k, (b,i)]  (+ pos-emb fold
     via a 3rd accumulation pass with identity rhs).
  Evicted to two bf16 copies: uh_di [n, b, d, i] and uh_id [n, b, i, d]
  so both routing contractions read contiguously.

  Routing iteration 1 is folded to the host: c1 = mask/128 is input-
  independent, so outputs1 = squash(s1) and the first agreement
  b2 = o1 . u_hat == uT^T @ w1 + peB1 with w1/peB1 computed on host.

  Iterations 2-3 on device: softmax over n done in [i, n] layout (fused
  Exp+sum on ACT), PE transposes to flip layouts, contractions as
  DVE/GPSIMD multiply + reduce in the matching contiguous layout.
"""

import numpy as np
import ml_dtypes

import concourse.bass as bass
import concourse.bacc as bacc
import concourse.tile as tile
from concourse import mybir
from concourse.bass_utils import run_bass_kernel_spmd

B, S, IND, N, D = 32, 128, 256, 128, 64
DEBUG = False
STAGE = 4
NCORES = 8
NB = B // NCORES  # batches per core
EPS = 1e-7
BF16 = mybir.dt.bfloat16
F32 = mybir.dt.float32
AF = mybir.ActivationFunctionType
ALU = mybir.AluOpType
AX = mybir.AxisListType
bf = ml_dtypes.bfloat16


def _pe_table(s_, d_):
    pos = np.arange(s_, dtype=np.float32)[:, None]
    inv = (1.0 / np.power(np.float32(10000.0),
                          (2.0 * np.arange(d_ // 2, dtype=np.float32)) / np.float32(d_))
           ).astype(np.float32)
    ang = pos * inv[None, :]
    return np.stack([np.sin(ang), np.cos(ang)], axis=-1).reshape(s_, d_).astype(np.float32)


def _squash_np(s):
    ss = np.sum(s * s, axis=-1, keepdims=True)
    return (ss / (1.0 + ss) / np.sqrt(ss + EPS)) * s


def _build_device():
    nc = bacc.Bacc("TRN2", target_bir_lowering=False)

    kmatp = nc.dram_tensor("kmatp", [128, 2, 64, 128], BF16, kind="ExternalInput")
    pe2p = nc.dram_tensor("pe2p", [128, 64, 128], BF16, kind="ExternalInput")
    i4 = nc.dram_tensor("i4", [128, NB, 128], BF16, kind="ExternalInput")
    ut = nc.dram_tensor("ut", [128, 2, NB, 128], BF16, kind="ExternalInput")
    utf = nc.dram_tensor("utf", [128, 2, NB, 128], F32, kind="ExternalInput")
    w1tf = nc.dram_tensor("w1tf", [128, 2, NB, 128], F32, kind="ExternalInput")
    peb1t = nc.dram_tensor("peb1t", [128, NB, 128], F32, kind="ExternalInput")
    mt = nc.dram_tensor("mt", [128, NB], F32, kind="ExternalInput")
    identb = nc.dram_tensor("identb", [128, 128], BF16, kind="ExternalInput")
    identf = nc.dram_tensor("identf", [128, 128], F32, kind="ExternalInput")
    outd = nc.dram_tensor("out", [NB, 128, D], F32, kind="ExternalOutput")
    if DEBUG:
        dbg_b2t = nc.dram_tensor("dbg_b2t", [128, 128], F32, kind="ExternalOutput")
        dbg_c2 = nc.dram_tensor("dbg_c2", [128, 128], F32, kind="ExternalOutput")
        dbg_pre2 = nc.dram_tensor("dbg_pre2", [128, D], F32, kind="ExternalOutput")
        dbg_b3 = nc.dram_tensor("dbg_b3", [128, 128], F32, kind="ExternalOutput")
        dbg_uh = nc.dram_tensor("dbg_uh", [128, 128, D], BF16, kind="ExternalOutput")

    with tile.TileContext(nc) as tc:
        with (
            tc.tile_pool(name="wrt", bufs=1) as wrt,
            tc.tile_pool(name="uhp", bufs=1) as uhp,
            tc.tile_pool(name="prt", bufs=1, space="PSUM") as prt,
        ):
            ut_t = wrt.tile([128, 2, NB, 128], BF16)
            utf_t = wrt.tile([128, 2, NB, 128], F32)
            w1tf_t = wrt.tile([128, 2, NB, 128], F32)
            peb1_t = wrt.tile([128, NB, 128], F32)
            mt_t = wrt.tile([128, NB], F32)
            idb_t = wrt.tile([128, 128], BF16)
            idf_t = wrt.tile([128, 128], F32)
            ostage = wrt.tile([128, NB, D], F32)
            eps_t = wrt.tile([128, 1], F32)
            nc.vector.memset(eps_t[:], EPS)
            nc.sync.dma_start(out=ut_t[:], in_=ut[:])
            nc.sync.dma_start(out=utf_t[:], in_=utf[:])
            nc.sync.dma_start(out=w1tf_t[:], in_=w1tf[:])
            nc.sync.dma_start(out=peb1_t[:], in_=peb1t[:])
            nc.sync.dma_start(out=mt_t[:], in_=mt[:])
            nc.sync.dma_start(out=idb_t[:], in_=identb[:])
            nc.sync.dma_start(out=idf_t[:], in_=identf[:])

            uh_di = uhp.tile([128, NB, D, 128], BF16)  # [n, b, d, i]
            uh_id = uhp.tile([128, NB, 128, D], BF16)  # [n, b, i, d]

            # ---------------- projection ----------------
            with (
                tc.tile_pool(name="wproj", bufs=1) as wproj,
                tc.tile_pool(name="pproj", bufs=5, space="PSUM") as pproj,
            ):
                km_t = wproj.tile([128, 2, 64, 128], BF16)
                pe_t = wproj.tile([128, 64, 128], BF16)
                i4_t = wproj.tile([128, NB, 128], BF16)
                nc.sync.dma_start(out=i4_t[:], in_=i4[:])
                # load in d-slabs so the d-loop can start early
                for d0 in range(0, 64, 8):
                    nc.sync.dma_start(out=km_t[:, :, d0:d0 + 8, :],
                                      in_=kmatp[:, :, d0:d0 + 8, :])
                    nc.sync.dma_start(out=pe_t[:, d0:d0 + 8, :],
                                      in_=pe2p[:, d0:d0 + 8, :])

                for d in range(D):
                    ps = pproj.tile([128, NB, 128], F32, tag="ps")
                    nc.tensor.matmul(ps[:], km_t[:, 0, d, :], ut_t[:, 0],
                                     start=True, stop=False)
                    nc.tensor.matmul(ps[:], km_t[:, 1, d, :], ut_t[:, 1],
                                     start=False, stop=False)
                    nc.tensor.matmul(ps[:], pe_t[:, d, :], i4_t[:],
                                     start=False, stop=True)
                    # evictions: split both copies between ACT and DVE
                    e1, e2 = (nc.scalar, nc.vector) if d % 2 == 0 else (nc.vector, nc.scalar)
                    e1.copy(uh_di[:, :, d, :], ps[:]) if e1 is nc.scalar else e1.tensor_copy(uh_di[:, :, d, :], ps[:])
                    e2.copy(uh_id[:, :, :, d], ps[:]) if e2 is nc.scalar else e2.tensor_copy(uh_id[:, :, :, d], ps[:])

            # ---------------- routing ----------------
            with (
                tc.tile_pool(name="rbig", bufs=1) as rbig,
                tc.tile_pool(name="rsm", bufs=3) as rsm,
            ):
                def softmax_to_c(bT_ap, b):
                    """softmax over n (free axis) of bT [i, n], * mask, -> c [n, i] bf16."""
                    e = rsm.tile([128, 128], F32, tag="e")
                    den = rsm.tile([128, 1], F32, tag="den")
                    mx = rsm.tile([128, 1], F32, tag="mx")
                    nc.vector.tensor_reduce(mx[:], bT_ap, axis=AX.X, op=ALU.max)
                    nmx = rsm.tile([128, 1], F32, tag="nmx")
                    nc.vector.tensor_scalar_mul(nmx[:], mx[:], -1.0)
                    nc.scalar.activation(e[:], bT_ap, AF.Exp, bias=nmx[:], accum_out=den[:])
                    rden = rsm.tile([128, 1], F32, tag="rden")
                    nc.vector.reciprocal(rden[:], den[:])
                    rm = rsm.tile([128, 1], F32, tag="rm")
                    nc.vector.tensor_mul(rm[:], rden[:], mt_t[:, b:b + 1])
                    cT = rsm.tile([128, 128], BF16, tag="cT")
                    nc.vector.tensor_scalar_mul(cT[:], e[:], rm[:])
                    cps = prt.tile([128, 128], BF16, tag="cpsb")
                    nc.tensor.transpose(cps[:], cT[:], idb_t[:])
                    c = rsm.tile([128, 128], BF16, tag="c")
                    nc.vector.tensor_copy(c[:], cps[:])
                    return c

                def contract1(c_bf, b, pre_ap):
                    """pre[n, d] = sum_i c[n, i] * uh_di[n, b, d, i]"""
                    tmp = rbig.tile([128, D, 128], BF16, tag="tmp1")
                    cb = c_bf[:].unsqueeze(1).broadcast_to([128, D, 128])
                    nc.vector.tensor_mul(tmp[:], uh_di[:, b], cb)
                    nc.vector.tensor_reduce(pre_ap, tmp[:], axis=AX.X, op=ALU.add)

                def contract2(o_bf, b, bout_ap):
                    """bout[n, i] = sum_d o[n, d] * uh_id[n, b, i, d]"""
                    tmp = rbig.tile([128, 128, D], BF16, tag="tmp2")
                    ob = o_bf[:].unsqueeze(1).broadcast_to([128, 128, D])
                    nc.gpsimd.tensor_mul(tmp[:], uh_id[:, b], ob)
                    nc.vector.tensor_reduce(bout_ap, tmp[:], axis=AX.X, op=ALU.add)

                def squash_dev(pre, out_f32_ap=None, out_bf_ap=None):
                    sq = rsm.tile([128, D], F32, tag="sq")
                    ss = rsm.tile([128, 1], F32, tag="ss")
                    nc.scalar.activation(sq[:], pre[:], AF.Square, accum_out=ss[:])
                    srt = rsm.tile([128, 1], F32, tag="srt")
                    nc.scalar.activation(srt[:], ss[:], AF.Sqrt, bias=eps_t[:])
                    ssp = rsm.tile([128, 1], F32, tag="ssp")
                    nc.vector.tensor_scalar_add(ssp[:], ss[:], 1.0)
                    dn = rsm.tile([128, 1], F32, tag="dn")
                    nc.vector.tensor_mul(dn[:], srt[:], ssp[:])
                    rcp = rsm.tile([128, 1], F32, tag="rcp")
                    nc.vector.reciprocal(rcp[:], dn[:])
                    scl = rsm.tile([128, 1], F32, tag="scl")
                    nc.vector.tensor_mul(scl[:], ss[:], rcp[:])
                    if out_f32_ap is not None:
                        nc.vector.tensor_scalar_mul(out_f32_ap, pre[:], scl[:])
                    if out_bf_ap is not None:
                        nc.vector.tensor_scalar_mul(out_bf_ap, pre[:], scl[:])

                for b in range(NB):
                    if STAGE < 2:
                        continue
                    # iter1 agreement via host-side w1: b2T[i, n]
                    bps = prt.tile([128, 128], F32, tag="bps")
                    nc.tensor.matmul(bps[:], utf_t[:, 0, b, :], w1tf_t[:, 0, b, :],
                                     start=True, stop=False)
                    nc.tensor.matmul(bps[:], utf_t[:, 1, b, :], w1tf_t[:, 1, b, :],
                                     start=False, stop=True)
                    b2T = rsm.tile([128, 128], F32, tag="b2T")
                    nc.vector.tensor_add(b2T[:], bps[:], peb1_t[:, b, :])
                    if DEBUG and b == 0:
                        nc.sync.dma_start(out=dbg_b2t[:], in_=b2T[:])

                    # iter 2
                    c2 = softmax_to_c(b2T[:], b)
                    if STAGE < 3:
                        continue
                    pre2 = rsm.tile([128, D], F32, tag="pre")
                    contract1(c2, b, pre2[:])
                    if DEBUG and b == 0:
                        c2f = rsm.tile([128, 128], F32, tag="c2f")
                        nc.vector.tensor_copy(c2f[:], c2[:])
                        nc.sync.dma_start(out=dbg_c2[:], in_=c2f[:])
                        nc.sync.dma_start(out=dbg_pre2[:], in_=pre2[:])
                        nc.sync.dma_start(out=dbg_uh[:], in_=uh_id[:, 0])
                    o2b = rsm.tile([128, D], BF16, tag="ob")
                    squash_dev(pre2, out_bf_ap=o2b[:])
                    if STAGE < 4:
                        continue
                    b3 = rsm.tile([128, 128], F32, tag="b3")
                    contract2(o2b, b, b3[:])
                    if DEBUG and b == 0:
                        nc.sync.dma_start(out=dbg_b3[:], in_=b3[:])

                    # iter 3
                    b3ps = prt.tile([128, 128], F32, tag="cpsf")
                    nc.tensor.transpose(b3ps[:], b3[:], idf_t[:])
                    c3 = softmax_to_c(b3ps[:], b)
                    pre3 = rsm.tile([128, D], F32, tag="pre")
                    contract1(c3, b, pre3[:])
                    squash_dev(pre3, out_f32_ap=ostage[:, b, :])
                    nc.sync.dma_start(out=outd[b], in_=ostage[:, b, :])

    nc.finalize()
    return nc


_NC_CACHE = None


def _host_prep(u_vecs, mask, W):
    """Host-side tensor preparation shared across cores + per-core shards."""
    pe1 = _pe_table(N, D)                      # [n, d]
    pe2 = _pe_table(S, N * D).reshape(S, N, D)  # [i, n, d]
    # Kmat[in, n, d] = W[0, in, d] + pe1[n, d]
    kmat = (W[0][:, None, :] + pe1[None, :, :]).astype(np.float32)  # [256, n, d]

    # iteration-1 shortcut (c1 = mask/128):
    mu = np.einsum('bi,biI->bI', mask, u_vecs)          # [B, 256]
    s1 = (np.einsum('bI,Ind->bnd', mu, kmat)
          + np.einsum('bi,ind->bnd', mask, pe2)) / np.float32(N)
    o1 = _squash_np(s1.astype(np.float32))              # [B, n, d]
    w1 = np.einsum('Ind,bnd->bnI', kmat, o1)            # [B, n, 256]
    peb1 = np.einsum('ind,bnd->ibn', pe2, o1)           # [i, B, n]

    kmat_h = np.ascontiguousarray(
        kmat.reshape(2, 128, N, D).transpose(1, 0, 3, 2)).astype(bf)  # [p, k, d, n]
    pe2_h = np.ascontiguousarray(pe2.transpose(0, 2, 1)).astype(bf)   # [i, d, n]
    i4_h = np.ascontiguousarray(
        np.broadcast_to(np.eye(128, dtype=np.float32)[:, None, :],
                        (128, NB, 128))).astype(bf)
    identb_h = np.eye(128, dtype=np.float32).astype(bf)
    identf_h = np.eye(128, dtype=np.float32)

    shared = dict(kmatp=kmat_h, pe2p=pe2_h, i4=i4_h,
                  identb=identb_h, identf=identf_h)

    in_maps = []
    for c in range(NCORES):
        sl = slice(c * NB, (c + 1) * NB)
        u_c = u_vecs[sl]                                 # [NB, i, in]
        utf_h = np.ascontiguousarray(
            u_c.transpose(2, 0, 1).reshape(2, 128, NB, 128)
               .transpose(1, 0, 2, 3)).astype(np.float32)  # [p, k, b, i]
        ut_h = utf_h.astype(bf)
        w1_c = w1[sl]                                    # [NB, n, in]
        w1tf_h = np.ascontiguousarray(
            w1_c.transpose(2, 0, 1).reshape(2, 128, NB, 128)
                .transpose(1, 0, 2, 3)).astype(np.float32)  # [p, k, b, n]
        peb1_h = np.ascontiguousarray(peb1[:, sl, :]).astype(np.float32)
        mt_h = np.ascontiguousarray(mask[sl].T).astype(np.float32)  # [i, b]
        m = dict(shared)
        m.update(ut=ut_h, utf=utf_h, w1tf=w1tf_h, peb1t=peb1_h, mt=mt_h)
        in_maps.append(m)
    return in_maps


def kernel(u_vecs, mask, W):
    global _NC_CACHE
    u_vecs = np.asarray(u_vecs, dtype=np.float32)
    mask = np.asarray(mask, dtype=np.float32)
    W = np.asarray(W, dtype=np.float32)

    in_maps = _host_prep(u_vecs, mask, W)
    if _NC_CACHE is None:
        _NC_CACHE = _build_device()
    res = run_bass_kernel_spmd(_NC_CACHE, in_maps, core_ids=list(range(NCORES)))
    outs = [np.asarray(r["out"], dtype=np.float32) for r in res.results]
    return np.concatenate(outs, axis=0)
